# revision 1
# baseline (speedup 1.0000x reference)
"""Trainium2 Bass kernel for BERT factorized attention.

Reference math (per batch b, head h, S=4096, H=1024, NH=16, HD=64):
    q = x @ Wq + bq ; k = x @ Wk + bk ; v = x @ Wv + bv
    s_probs = softmax_S(qT_head)            # [HD, S]
    c_probs = softmax_HD(k_head)            # [S, HD]
    s_ctx   = s_probs @ v_head              # [HD, HD]
    out     = c_probs @ s_ctx               # [S, HD]

Kernel strategy (one batch element per NeuronCore, 8 cores, no collectives):
  - PE-transpose X -> XT per 512-row stripe; projections via float32r
    (TF32-like) matmuls at full PE rate (or bf16 when proj_dt=bf16).
  - Softmax denominators come from augmented matmuls:
      phase2: psum[d, 0:65] = EQ_chunk.T @ [V | 1]   accumulated over S
      den:    psum[s, 0:2]  = EK_tile.T @ ones-blockdiag  (c-softmax denom)
    (exp without max-subtraction is safe: q,k ~ N(0,1)).
  - EKT (exp of K-transposed) spills to DRAM scratch between pass A and B.
  - Pass B: out_head = (EKT_chunk.T @ s_ctx) * recip(denom) via one
    broadcast tensor_tensor multiply per head-pair.
"""

import sys

sys.path.insert(0, "/opt/trn_rl_repo")

import contextlib
from contextlib import ExitStack

import numpy as np

import concourse.bass as bass
import concourse.mybir as mybir
import concourse.tile as tile
from concourse import bacc, bass_utils
from concourse.masks import make_identity

F32 = mybir.dt.float32
F32R = mybir.dt.float32r
BF16 = mybir.dt.bfloat16

B, S, H = 8, 4096, 1024
NH, HD = 16, 64
STRIPE = 512
CPS = STRIPE // 128  # chunks per stripe
KT = H // 128  # contraction tiles
NP = NH // 2  # head pairs

EXPF = mybir.ActivationFunctionType.Exp
COPYF = mybir.ActivationFunctionType.Copy


def _bcast(ap_2d, n):
    """[p, c] AP -> [p, c, n] with step-0 broadcast on the last dim."""
    return bass.AP(
        tensor=ap_2d.tensor,
        offset=ap_2d.offset,
        ap=[ap_2d.ap[0], ap_2d.ap[1], [0, n]],
    )


def build_kernel(seq_len=S, with_bias=False, loop_n=None, proj_dt=F32R):
    """Build + compile the single-core program (SPMD across 8 cores)."""
    s = seq_len
    n_stripes = s // STRIPE
    n_chunks = s // 128

    nc = bacc.Bacc("TRN2", target_bir_lowering=False, debug=False, num_devices=8)

    x_d = nc.dram_tensor("x", [s, H], F32, kind="ExternalInput").ap()
    m_d = nc.dram_tensor("mask", [s], F32, kind="ExternalInput").ap()
    wq_d = nc.dram_tensor("wq", [H, H], F32, kind="ExternalInput").ap()
    wk_d = nc.dram_tensor("wk", [H, H], F32, kind="ExternalInput").ap()
    wv_d = nc.dram_tensor("wv", [H, H], F32, kind="ExternalInput").ap()
    if with_bias:
        bq_d = nc.dram_tensor("bq", [H], F32, kind="ExternalInput").ap()
        bk_d = nc.dram_tensor("bk", [H], F32, kind="ExternalInput").ap()
        bv_d = nc.dram_tensor("bv", [H], F32, kind="ExternalInput").ap()
    out_d = nc.dram_tensor("out", [s, H], F32, kind="ExternalOutput").ap()
    ekt_ds = [
        nc.dram_tensor(f"ekts{i}", [KT, 128, STRIPE], BF16, kind="Internal").ap()
        for i in range(n_stripes)
    ]

    with tile.TileContext(nc) as tc:
        with ExitStack() as ctx:
            singles = ctx.enter_context(tc.tile_pool(name="singles", bufs=1))
            xpool = ctx.enter_context(tc.tile_pool(name="xpool", bufs=2))
            xtpool = ctx.enter_context(tc.tile_pool(name="xtpool", bufs=2))
            eqpool = ctx.enter_context(tc.tile_pool(name="eqpool", bufs=3))
            vapool = ctx.enter_context(tc.tile_pool(name="vapool", bufs=3))
            ekpool = ctx.enter_context(tc.tile_pool(name="ekpool", bufs=3))
            ecpool = ctx.enter_context(tc.tile_pool(name="ecpool", bufs=2))
            opool = ctx.enter_context(tc.tile_pool(name="opool", bufs=3))
            small = ctx.enter_context(tc.tile_pool(name="small", bufs=4))
            # PSUM (8 banks): tp 2 + proj 2 + kt 2 + p2 2 (shared with den)
            tp = ctx.enter_context(tc.tile_pool(name="tp", bufs=2, space="PSUM"))
            proj = ctx.enter_context(tc.tile_pool(name="proj", bufs=2, space="PSUM"))
            ktp = ctx.enter_context(tc.tile_pool(name="ktp", bufs=2, space="PSUM"))
            p2p = ctx.enter_context(tc.tile_pool(name="p2p", bufs=2, space="PSUM"))
            dpp = p2p

            ident = singles.tile([128, 128], F32)
            make_identity(nc, ident)

            mask_sb = singles.tile([128, n_chunks], F32)
            nc.gpsimd.dma_start(out=mask_sb, in_=m_d.rearrange("(c p) -> p c", p=128))

            # weights: DMA fp32 staging chunks, round to proj dtype
            w_r = {}
            for name, wd in (("wq", wq_d), ("wv", wv_d), ("wk", wk_d)):
                wr = singles.tile([128, KT, H], proj_dt, tag=f"{name}_r")
                w_r[name] = wr
                for k in range(KT):
                    st = opool.tile([128, H], F32, tag="ob")
                    nc.gpsimd.dma_start(out=st, in_=wd[k * 128 : (k + 1) * 128, :])
                    nc.scalar.activation(wr[:, k, :], st, COPYF)
            wq_r, wk_r, wv_r = w_r["wq"], w_r["wk"], w_r["wv"]

            if with_bias:
                bqb = singles.tile([128, H], F32)
                bvb = singles.tile([128, H], F32)
                for bt, bd in ((bqb, bq_d), (bvb, bv_d)):
                    src = bass.AP(
                        tensor=bd.tensor, offset=bd.offset, ap=[[0, 128], bd.ap[0]]
                    )
                    nc.sync.dma_start(out=bt, in_=src)
                bkc = singles.tile([128, KT], F32)
                nc.sync.dma_start(out=bkc, in_=bk_d.rearrange("(t p) -> p t", p=128))

            acc = singles.tile([128, NP, 130], F32)
            sctf = singles.tile([128, NP, 128], F32)
            sctx = singles.tile([128, NP, 128], BF16)
            sctl = singles.tile([128, NP, 128], BF16)
            denoms = singles.tile([128, n_chunks, 16], F32)
            # f32r constants (memset of f32r is rejected by walrus; use
            # fp32->f32r tensor_copy from fp32 constant tiles)
            ones16 = singles.tile([128, 16, 1], F32)
            nc.vector.memset(ones16, 1.0)
            z128 = singles.tile([128, 128], F32)
            nc.vector.memset(z128, 0.0)
            ones2 = singles.tile([128, 2], BF16)
            nc.vector.tensor_copy(ones2, z128[:, 0:2])
            nc.vector.tensor_copy(ones2[0:64, 0:1], ones16[0:64, 0, :])
            nc.vector.tensor_copy(ones2[64:128, 1:2], ones16[64:128, 0, :])

            loop_cm = tc.For_i(0, loop_n, 1) if loop_n else contextlib.nullcontext()
            with loop_cm:
                nc.vector.memset(acc, 0.0)

                # ---------------- PASS A ----------------
                for st_i in range(n_stripes):
                    s0 = st_i * STRIPE
                    xt = xtpool.tile([128, KT, STRIPE], proj_dt)
                    for c in range(CPS):
                        xc = xpool.tile([128, H], F32)
                        nc.sync.dma_start(
                            out=xc, in_=x_d[s0 + c * 128 : s0 + (c + 1) * 128, :]
                        )
                        for g in range(KT // 4):
                            pt = tp.tile([128, 4, 128], F32)
                            for kk in range(4):
                                k = g * 4 + kk
                                nc.tensor.transpose(
                                    pt[:, kk, :],
                                    xc[:, k * 128 : (k + 1) * 128],
                                    ident,
                                )
                            nc.vector.tensor_copy(
                                xt[:, g * 4 : (g + 1) * 4, c * 128 : (c + 1) * 128],
                                pt,
                            )

                    # Q,V projections + phase 2 in chunk pairs
                    for cp in range(CPS // 2):
                        eqs, vas = [], []
                        for c in (2 * cp, 2 * cp + 1):
                            sc = st_i * CPS + c
                            cs = slice(c * 128, (c + 1) * 128)
                            eqc = eqpool.tile([128, H], F32R, tag="eq")
                            vac = vapool.tile([128, NH, 65], F32R, tag="va")
                            eqs.append(eqc)
                            vas.append(vac)
                            mb = mask_sb[:, sc : sc + 1]
                            for half in range(2):
                                hs = slice(half * 512, (half + 1) * 512)
                                pq = proj.tile([128, 512], F32, tag="proj")
                                for k in range(KT):
                                    nc.tensor.matmul(
                                        pq,
                                        xt[:, k, cs],
                                        wq_r[:, k, hs],
                                        start=k == 0,
                                        stop=k == KT - 1,
                                    )
                                if with_bias:
                                    nc.vector.tensor_add(pq, pq, bqb[:, hs])
                                nc.scalar.activation(eqc[:, hs], pq, EXPF, bias=mb)
                            for half in range(2):
                                hs = slice(half * 512, (half + 1) * 512)
                                pv = proj.tile([128, 512], F32, tag="proj")
                                for k in range(KT):
                                    nc.tensor.matmul(
                                        pv,
                                        xt[:, k, cs],
                                        wv_r[:, k, hs],
                                        start=k == 0,
                                        stop=k == KT - 1,
                                    )
                                dst = vac[:, half * 8 : (half + 1) * 8, 0:64]
                                src = pv[:].rearrange("p (h e) -> p h e", e=64)
                                if with_bias:
                                    nc.vector.tensor_add(
                                        dst,
                                        src,
                                        bvb[:, hs].rearrange(
                                            "p (h e) -> p h e", e=64
                                        ),
                                    )
                                else:
                                    nc.scalar.activation(dst, src, COPYF)
                            nc.vector.tensor_copy(vac[:, :, 64:65], ones16)

                        for hp in range(NP):
                            p2 = p2p.tile([128, 130], F32, tag="p2")
                            for j in range(2):
                                nc.tensor.matmul(
                                    p2,
                                    eqs[j][:, hp * 128 : (hp + 1) * 128],
                                    vas[j][:, hp * 2 : hp * 2 + 2, :],
                                    start=j == 0,
                                    stop=j == 1,
                                )
                            nc.vector.tensor_add(acc[:, hp, :], acc[:, hp, :], p2)

                    # K-transposed projection -> exp -> denoms + DRAM scratch
                    for t in range(KT):
                        pk = ktp.tile([128, 512], F32, tag="pk")
                        for k in range(KT):
                            nc.tensor.matmul(
                                pk,
                                wk_r[:, k, t * 128 : (t + 1) * 128],
                                xt[:, k, :],
                                start=k == 0,
                                stop=k == KT - 1,
                            )
                        ek = ekpool.tile([128, STRIPE], BF16)
                        if with_bias:
                            nc.scalar.activation(
                                ek, pk, EXPF, bias=bkc[:, t : t + 1]
                            )
                        else:
                            nc.scalar.activation(ek, pk, EXPF)
                        nc.gpsimd.dma_start(out=ekt_ds[st_i][t], in_=ek)
                        dps = dpp.tile([128, CPS, 2], F32, tag="p2")
                        for c in range(CPS):
                            nc.tensor.matmul(
                                dps[:, c, :],
                                ek[:, c * 128 : (c + 1) * 128],
                                ones2,
                                start=True,
                                stop=True,
                            )
                        nc.vector.tensor_copy(
                            denoms[
                                :, st_i * CPS : (st_i + 1) * CPS, 2 * t : 2 * t + 2
                            ],
                            dps,
                        )

                # ---------------- finalize s_ctx (bf16 hi/lo split) ------
                nc.vector.memset(sctf, 0.0)
                for hp in range(NP):
                    r0 = small.tile([64, 1], F32, tag="r0")
                    r1 = small.tile([64, 1], F32, tag="r1")
                    nc.vector.reciprocal(r0, acc[0:64, hp, 64:65])
                    nc.vector.reciprocal(r1, acc[64:128, hp, 129:130])
                    nc.vector.tensor_scalar_mul(
                        sctf[0:64, hp, 0:64], acc[0:64, hp, 0:64], r0
                    )
                    nc.vector.tensor_scalar_mul(
                        sctf[64:128, hp, 64:128], acc[64:128, hp, 65:129], r1
                    )
                nc.vector.tensor_copy(sctx, sctf)
                nc.vector.tensor_tensor(
                    out=sctl, in0=sctf, in1=sctx, op=mybir.AluOpType.subtract
                )

                # ---------------- PASS B ----------------
                for cc in range(n_chunks):
                    if cc % CPS == 0:
                        ec = ecpool.tile([128, KT, STRIPE], BF16)
                        for t in range(KT):
                            nc.sync.dma_start(out=ec[:, t, :], in_=ekt_ds[cc // CPS][t])
                    co = cc % CPS
                    r16 = small.tile([128, 16], F32, tag="r16")
                    nc.vector.reciprocal(r16, denoms[:, cc, :])
                    ob = opool.tile([128, H], F32)
                    for hp in range(NP):
                        if hp % 2 == 0:
                            p3 = proj.tile([128, 128], F32, tag="proj")
                        else:
                            p3 = ktp.tile([128, 128], F32, tag="pk")
                        nc.tensor.matmul(
                            p3,
                            ec[:, hp, co * 128 : (co + 1) * 128],
                            sctx[:, hp, :],
                            start=True,
                            stop=False,
                        )
                        nc.tensor.matmul(
                            p3,
                            ec[:, hp, co * 128 : (co + 1) * 128],
                            sctl[:, hp, :],
                            start=False,
                            stop=True,
                        )
                        dst = ob[:, hp * 128 : (hp + 1) * 128]
                        if hp % 2 == 0:
                            nc.vector.tensor_copy(dst, p3)
                        else:
                            nc.scalar.activation(dst, p3, COPYF)
                    obv = ob[:].rearrange("p (h e) -> p h e", e=64)
                    nc.vector.tensor_tensor(
                        out=obv, in0=obv, in1=_bcast(r16, 64),
                        op=mybir.AluOpType.mult,
                    )
                    nc.gpsimd.dma_start(
                        out=out_d[cc * 128 : (cc + 1) * 128, :], in_=ob
                    )

    nc.compile()
    return nc


_CACHE = {}


def _get_nc(seq_len, with_bias):
    key = (seq_len, with_bias)
    if key not in _CACHE:
        _CACHE[key] = build_kernel(seq_len, with_bias)
    return _CACHE[key]


def kernel(hidden_states, attention_mask, Wq, bq, Wk, bk, Wv, bv):
    hidden_states = np.asarray(hidden_states, dtype=np.float32)
    attention_mask = np.asarray(attention_mask, dtype=np.float32)
    Wq = np.asarray(Wq, dtype=np.float32)
    Wk = np.asarray(Wk, dtype=np.float32)
    Wv = np.asarray(Wv, dtype=np.float32)
    bq = np.asarray(bq, dtype=np.float32)
    bk = np.asarray(bk, dtype=np.float32)
    bv = np.asarray(bv, dtype=np.float32)
    b, s, h = hidden_states.shape
    with_bias = bool(bq.any() or bk.any() or bv.any())
    nc = _get_nc(s, with_bias)

    mask = attention_mask.reshape(b, s)
    in_maps = []
    for i in range(b):
        m = {
            "x": np.ascontiguousarray(hidden_states[i]),
            "mask": np.ascontiguousarray(mask[i]),
            "wq": Wq,
            "wk": Wk,
            "wv": Wv,
        }
        if with_bias:
            m.update({"bq": bq, "bk": bk, "bv": bv})
        in_maps.append(m)

    res = bass_utils.run_bass_kernel_spmd(nc, in_maps, core_ids=list(range(b)))
    return np.stack([res.results[i]["out"] for i in range(b)], axis=0)



# revision 6
# speedup vs baseline: 4.0872x; 4.0872x over previous
"""Trainium2 Bass kernel for BERT factorized attention.

Reference math (per batch b, head h, S=4096, H=1024, NH=16, HD=64):
    q = x @ Wq + bq ; k = x @ Wk + bk ; v = x @ Wv + bv
    s_probs = softmax_S(qT_head)            # [HD, S]
    c_probs = softmax_HD(k_head)            # [S, HD]
    s_ctx   = s_probs @ v_head              # [HD, HD]
    out     = c_probs @ s_ctx               # [S, HD]

Kernel strategy (one batch element per NeuronCore, 8 cores, no collectives):
  - PE-transpose X -> XT per 512-row stripe (f32); the PSUM->SBUF drains
    produce XT in bf16 (DVE) and fp8-e4m3 (Pool) copies.
  - Q,V projections run as fp8 DoubleRow matmuls (256-deep contraction,
    2x PE rate) against x32-prescaled fp8 weights; the 1/32 rescale is
    folded into the exp / copy activation. K projection stays bf16
    (c-softmax is precision-sensitive; fp8 K fails the 2e-2 gate).
  - Softmax denominators come from augmented matmuls:
      phase2: psum[d, 0:65] = EQ_chunk.T @ [V | 1]   accumulated over S
      den:    psum[s, 0:2]  = EK_tile.T @ ones-blockdiag  (c-softmax denom)
    (exp without max-subtraction is safe: q,k ~ N(0,1)).
  - EKT (exp of K-transposed, bf16) spills to DRAM scratch between passes.
  - Pass B: one bf16 matmul per head-pair; the c-softmax normalization
    multiply is fused into the PSUM->SBUF drain (split DVE/Pool).
"""

import sys

sys.path.insert(0, "/opt/trn_rl_repo")

import contextlib
from contextlib import ExitStack

import numpy as np

import concourse.bass as bass
import concourse.mybir as mybir
import concourse.tile as tile
from concourse import bacc, bass_utils
from concourse.masks import make_identity

F32 = mybir.dt.float32
BF16 = mybir.dt.bfloat16
FP8 = mybir.dt.float8e4

B, S, H = 8, 4096, 1024
NH, HD = 16, 64
STRIPE = 512
CPS = STRIPE // 128  # chunks per stripe
KT = H // 128  # contraction tiles
NP = NH // 2  # head pairs
WSCALE = 32.0  # fp8 weight prescale (power of 2; undone in activations)

EXPF = mybir.ActivationFunctionType.Exp
COPYF = mybir.ActivationFunctionType.Copy
DR = mybir.MatmulPerfMode.DoubleRow


def _bcast(ap_2d, n):
    """[p, c] AP -> [p, c, n] with step-0 broadcast on the last dim."""
    return bass.AP(
        tensor=ap_2d.tensor,
        offset=ap_2d.offset,
        ap=[ap_2d.ap[0], ap_2d.ap[1], [0, n]],
    )


def build_kernel(seq_len=S, with_bias=False, loop_n=None):
    """Build + compile the single-core program (SPMD across 8 cores)."""
    s = seq_len
    n_stripes = s // STRIPE
    n_chunks = s // 128

    nc = bacc.Bacc("TRN2", target_bir_lowering=False, debug=False, num_devices=8)

    x_d = nc.dram_tensor("x", [s, H], F32, kind="ExternalInput").ap()
    m_d = nc.dram_tensor("mask", [s], F32, kind="ExternalInput").ap()
    wq_d = nc.dram_tensor("wq", [H, H], F32, kind="ExternalInput").ap()
    wk_d = nc.dram_tensor("wk", [H, H], F32, kind="ExternalInput").ap()
    wv_d = nc.dram_tensor("wv", [H, H], F32, kind="ExternalInput").ap()
    if with_bias:
        bq_d = nc.dram_tensor("bq", [H], F32, kind="ExternalInput").ap()
        bk_d = nc.dram_tensor("bk", [H], F32, kind="ExternalInput").ap()
        bv_d = nc.dram_tensor("bv", [H], F32, kind="ExternalInput").ap()
    out_d = nc.dram_tensor("out", [s, H], F32, kind="ExternalOutput").ap()
    ekt_ds = [
        nc.dram_tensor(f"ekts{i}", [KT, 128, STRIPE], BF16, kind="Internal").ap()
        for i in range(n_stripes)
    ]

    with tile.TileContext(nc) as tc:
        with ExitStack() as ctx:
            singles = ctx.enter_context(tc.tile_pool(name="singles", bufs=1))
            xpool = ctx.enter_context(tc.tile_pool(name="xpool", bufs=2))
            xtbpool = ctx.enter_context(tc.tile_pool(name="xtbpool", bufs=2))
            xt8pool = ctx.enter_context(tc.tile_pool(name="xt8pool", bufs=2))
            eqpool = ctx.enter_context(tc.tile_pool(name="eqpool", bufs=3))
            vapool = ctx.enter_context(tc.tile_pool(name="vapool", bufs=3))
            ekpool = ctx.enter_context(tc.tile_pool(name="ekpool", bufs=3))
            ecpool = ctx.enter_context(tc.tile_pool(name="ecpool", bufs=2))
            opool = ctx.enter_context(tc.tile_pool(name="opool", bufs=3))
            small = ctx.enter_context(tc.tile_pool(name="small", bufs=4))
            # PSUM (8 banks): tp 2 + proj 3 + ph2 2 + den 1
            tp = ctx.enter_context(tc.tile_pool(name="tp", bufs=2, space="PSUM"))
            proj = ctx.enter_context(tc.tile_pool(name="proj", bufs=3, space="PSUM"))
            ph2 = ctx.enter_context(tc.tile_pool(name="ph2", bufs=1, space="PSUM"))
            denp = ctx.enter_context(tc.tile_pool(name="denp", bufs=1, space="PSUM"))

            ident = singles.tile([128, 128], F32)
            make_identity(nc, ident)

            mask_sb = singles.tile([128, n_chunks], F32)
            nc.gpsimd.dma_start(out=mask_sb, in_=m_d.rearrange("(c p) -> p c", p=128))

            # weights: DMA fp32 staging chunks, convert to kernel dtypes.
            # wq/wv -> fp8 prescaled by 32 (undone in activations);
            # wk -> bf16.
            wq8 = singles.tile([128, KT, H], FP8, tag="wq8")
            wv8 = singles.tile([128, KT, H], FP8, tag="wv8")
            wkb = singles.tile([128, KT, H], BF16, tag="wkb")
            for wt, wd, sc in ((wq8, wq_d, WSCALE), (wv8, wv_d, WSCALE),
                               (wkb, wk_d, 1.0)):
                for k in range(KT):
                    st = opool.tile([128, H], F32, tag="ob")
                    nc.gpsimd.dma_start(out=st, in_=wd[k * 128 : (k + 1) * 128, :])
                    nc.scalar.activation(wt[:, k, :], st, COPYF, scale=sc)

            if with_bias:
                bqb = singles.tile([128, H], F32)
                bvb = singles.tile([128, H], F32)
                for bt, bd in ((bqb, bq_d), (bvb, bv_d)):
                    src = bass.AP(
                        tensor=bd.tensor, offset=bd.offset, ap=[[0, 128], bd.ap[0]]
                    )
                    st = opool.tile([128, H], F32, tag="ob")
                    nc.sync.dma_start(out=st, in_=src)
                    nc.scalar.activation(bt, st, COPYF, scale=WSCALE)
                bkc = singles.tile([128, KT], F32)
                nc.sync.dma_start(out=bkc, in_=bk_d.rearrange("(t p) -> p t", p=128))

            acc = singles.tile([128, NP, 130], F32)
            sct = singles.tile([128, NP, 128], BF16)
            denoms = singles.tile([128, n_chunks, 16], F32)
            ones2 = singles.tile([128, 2], BF16)
            nc.vector.memset(ones2, 0.0)
            nc.vector.memset(ones2[0:64, 0:1], 1.0)
            nc.vector.memset(ones2[64:128, 1:2], 1.0)

            loop_cm = tc.For_i(0, loop_n, 1) if loop_n else contextlib.nullcontext()
            with loop_cm:
                nc.vector.memset(acc, 0.0)

                # ---------------- PASS A ----------------
                for st_i in range(n_stripes):
                    s0 = st_i * STRIPE
                    xtb = xtbpool.tile([128, KT, STRIPE], BF16)
                    xt8 = xt8pool.tile([128, KT, STRIPE], FP8)
                    for c in range(CPS):
                        xc = xpool.tile([128, H], F32)
                        nc.sync.dma_start(
                            out=xc, in_=x_d[s0 + c * 128 : s0 + (c + 1) * 128, :]
                        )
                        for g in range(KT // 4):
                            pt = tp.tile([128, 4, 128], F32)
                            for kk in range(4):
                                k = g * 4 + kk
                                nc.tensor.transpose(
                                    pt[:, kk, :],
                                    xc[:, k * 128 : (k + 1) * 128],
                                    ident,
                                )
                            cs = slice(c * 128, (c + 1) * 128)
                            nc.vector.tensor_copy(
                                xtb[:, g * 4 : (g + 1) * 4, cs], pt
                            )
                            # gpsimd has no PSUM access; derive fp8 from bf16
                            nc.gpsimd.tensor_copy(
                                xt8[:, g * 4 : (g + 1) * 4, cs],
                                xtb[:, g * 4 : (g + 1) * 4, cs],
                            )

                    # Q,V fp8 DoubleRow projections + bf16 phase 2, chunk pairs
                    for cp in range(CPS // 2):
                        eqs, vas = [], []
                        for c in (2 * cp, 2 * cp + 1):
                            sc_i = st_i * CPS + c
                            cs = slice(c * 128, (c + 1) * 128)
                            eqc = eqpool.tile([128, H], BF16, tag="eq")
                            vac = vapool.tile([128, NH, 65], BF16, tag="va")
                            eqs.append(eqc)
                            vas.append(vac)
                            mb = mask_sb[:, sc_i : sc_i + 1]
                            for half in range(2):
                                hs = slice(half * 512, (half + 1) * 512)
                                pq = proj.tile([128, 512], F32, tag="proj")
                                for kp in range(KT // 2):
                                    ks = slice(2 * kp, 2 * kp + 2)
                                    nc.tensor.matmul(
                                        pq,
                                        xt8[:, ks, cs],
                                        wq8[:, ks, hs],
                                        start=kp == 0,
                                        stop=kp == KT // 2 - 1,
                                        perf_mode=DR,
                                    )
                                if with_bias:
                                    nc.vector.tensor_add(pq, pq, bqb[:, hs])
                                nc.scalar.activation(
                                    eqc[:, hs], pq, EXPF, bias=mb, scale=1.0 / WSCALE
                                )
                            nc.vector.memset(vac[:, :, 64:65], 1.0)
                            for half in range(2):
                                hs = slice(half * 512, (half + 1) * 512)
                                pv = proj.tile([128, 512], F32, tag="proj")
                                for kp in range(KT // 2):
                                    ks = slice(2 * kp, 2 * kp + 2)
                                    nc.tensor.matmul(
                                        pv,
                                        xt8[:, ks, cs],
                                        wv8[:, ks, hs],
                                        start=kp == 0,
                                        stop=kp == KT // 2 - 1,
                                        perf_mode=DR,
                                    )
                                if with_bias:
                                    nc.vector.tensor_add(pv, pv, bvb[:, hs])
                                dst = vac[:, half * 8 : (half + 1) * 8, 0:64]
                                src = pv[:].rearrange("p (h e) -> p h e", e=64)
                                nc.scalar.activation(
                                    dst, src, COPYF, scale=1.0 / WSCALE
                                )

                        for hpp in range(NP // 2):  # head-pair pairs -> 1 psum bank
                            p2 = ph2.tile(
                                [128, 2, 130], F32, tag=f"p2{hpp % 2}", name="p2"
                            )
                            for sub in range(2):
                                hp = 2 * hpp + sub
                                for j in range(2):
                                    nc.tensor.matmul(
                                        p2[:, sub, :],
                                        eqs[j][:, hp * 128 : (hp + 1) * 128],
                                        vas[j][:, hp * 2 : hp * 2 + 2, :],
                                        start=j == 0,
                                        stop=j == 1,
                                    )
                            nc.vector.tensor_add(
                                acc[:, 2 * hpp : 2 * hpp + 2, :],
                                acc[:, 2 * hpp : 2 * hpp + 2, :],
                                p2,
                            )

                    # K-transposed bf16 projection -> exp -> denoms + scratch
                    dps = denp.tile([128, CPS, 16], F32, tag="den", name="dps")
                    for t in range(KT):
                        pk = proj.tile([128, 512], F32, tag="proj", name="pk")
                        for k in range(KT):
                            nc.tensor.matmul(
                                pk,
                                wkb[:, k, t * 128 : (t + 1) * 128],
                                xtb[:, k, :],
                                start=k == 0,
                                stop=k == KT - 1,
                            )
                        ek = ekpool.tile([128, STRIPE], BF16)
                        if with_bias:
                            nc.scalar.activation(
                                ek, pk, EXPF, bias=bkc[:, t : t + 1]
                            )
                        else:
                            nc.scalar.activation(ek, pk, EXPF)
                        nc.gpsimd.dma_start(out=ekt_ds[st_i][t], in_=ek)
                        for c in range(CPS):
                            nc.tensor.matmul(
                                dps[:, c, 2 * t : 2 * t + 2],
                                ek[:, c * 128 : (c + 1) * 128],
                                ones2,
                                start=True,
                                stop=True,
                            )
                    nc.scalar.activation(
                        denoms[:, st_i * CPS : (st_i + 1) * CPS, :], dps, COPYF
                    )

                # ---------------- finalize s_ctx (single bf16) ----------
                nc.vector.memset(sct, 0.0)
                for hp in range(NP):
                    r0 = small.tile([64, 1], F32, tag="r0")
                    r1 = small.tile([64, 1], F32, tag="r1")
                    nc.vector.reciprocal(r0, acc[0:64, hp, 64:65])
                    nc.vector.reciprocal(r1, acc[64:128, hp, 129:130])
                    nc.vector.tensor_scalar_mul(
                        sct[0:64, hp, 0:64], acc[0:64, hp, 0:64], r0
                    )
                    nc.vector.tensor_scalar_mul(
                        sct[64:128, hp, 64:128], acc[64:128, hp, 65:129], r1
                    )

                # ---------------- PASS B ----------------
                for cc in range(n_chunks):
                    if cc % CPS == 0:
                        ec = ecpool.tile([128, KT, STRIPE], BF16)
                        for t in range(KT):
                            nc.sync.dma_start(
                                out=ec[:, t, :], in_=ekt_ds[cc // CPS][t]
                            )
                    co = cc % CPS
                    cos = slice(co * 128, (co + 1) * 128)
                    r16 = small.tile([128, 16], F32, tag="r16")
                    nc.vector.reciprocal(r16, denoms[:, cc, :])
                    ob = opool.tile([128, H], F32, tag="ob")
                    for grp in range(2):  # head-pairs 0-3 / 4-7
                        p3 = proj.tile([128, 4, 128], F32, tag="proj", name="p3")
                        for i in range(4):
                            hp = grp * 4 + i
                            nc.tensor.matmul(
                                p3[:, i, :],
                                ec[:, hp, cos],
                                sct[:, hp, :],
                                start=True,
                                stop=True,
                            )
                        obv = ob[:, grp * 512 : (grp + 1) * 512].rearrange(
                            "p (h e) -> p h e", e=64
                        )
                        p3v = p3[:].rearrange("p a (c e) -> p (a c) e", e=64)
                        rb = _bcast(r16[:, grp * 8 : (grp + 1) * 8], 64)
                        nc.vector.tensor_tensor(
                            out=obv, in0=p3v, in1=rb, op=mybir.AluOpType.mult
                        )
                    nc.sync.dma_start(
                        out=out_d[cc * 128 : (cc + 1) * 128, :], in_=ob
                    )

    nc.compile()
    return nc


_CACHE = {}


def _get_nc(seq_len, with_bias):
    key = (seq_len, with_bias)
    if key not in _CACHE:
        _CACHE[key] = build_kernel(seq_len, with_bias)
    return _CACHE[key]


def kernel(hidden_states, attention_mask, Wq, bq, Wk, bk, Wv, bv):
    hidden_states = np.asarray(hidden_states, dtype=np.float32)
    attention_mask = np.asarray(attention_mask, dtype=np.float32)
    Wq = np.asarray(Wq, dtype=np.float32)
    Wk = np.asarray(Wk, dtype=np.float32)
    Wv = np.asarray(Wv, dtype=np.float32)
    bq = np.asarray(bq, dtype=np.float32)
    bk = np.asarray(bk, dtype=np.float32)
    bv = np.asarray(bv, dtype=np.float32)
    b, s, h = hidden_states.shape
    with_bias = bool(bq.any() or bk.any() or bv.any())
    nc = _get_nc(s, with_bias)

    mask = attention_mask.reshape(b, s)
    in_maps = []
    for i in range(b):
        m = {
            "x": np.ascontiguousarray(hidden_states[i]),
            "mask": np.ascontiguousarray(mask[i]),
            "wq": Wq,
            "wk": Wk,
            "wv": Wv,
        }
        if with_bias:
            m.update({"bq": bq, "bk": bk, "bv": bv})
        in_maps.append(m)

    res = bass_utils.run_bass_kernel_spmd(nc, in_maps, core_ids=list(range(b)))
    return np.stack([res.results[i]["out"] for i in range(b)], axis=0)
